# revision 2
# baseline (speedup 1.0000x reference)
"""Trainium2 Bass kernel for a 2-layer LSTM agent (T=512, B=128, IN=544, H=512)
with per-step done-masking plus actor/critic heads.

Strategy: data-parallel over batch (B=128 -> 16 per core on 8 cores).
Everything is kept transposed on device (feature dim on partitions, batch on
the free dim):
  - bulk input projections A0 = W_ih0 @ x^T and A1 = W_ih1 @ h0^T as large
    matmuls (full PE utilization),
  - the serial recurrence per step only computes W_hh @ h^T (64 small
    matmuls/step) + fused elementwise gate math at full 128-lane width,
  - actor/critic heads as a final bulk matmul.
All activations/state transfers stay in bf16 for matmul speed; cell state c
stays fp32.
"""
import os
import numpy as np
import ml_dtypes

import concourse.bass as bass
import concourse.mybir as mybir
import concourse.tile as tile
from concourse.bass import ds
from concourse.bass_utils import run_bass_kernel_spmd

BF16 = mybir.dt.bfloat16
F32 = mybir.dt.float32
AF = mybir.ActivationFunctionType

NCORES = 8
B = 128
BL = B // NCORES          # 16 batch elements per core
H = 512
IN = 544
INP = 640                 # IN padded to 5*128
G4 = 4 * H                # 2048 gate rows
NM = G4 // 128            # 16 M-tiles
NK = H // 128             # 4 K-chunks for H
NKX = INP // 128          # 5 K-chunks for padded input
A = 18
AH = 19                   # actor+critic rows
TB = 16                   # timesteps per block

# ---------------------------------------------------------------------------
# Workaround for this walrus build: cap sem-waits per instruction at 1 by
# splitting excess waits onto NoOp prefix instructions on the same engine.
# ---------------------------------------------------------------------------
import orjson

_MAXW = 1


def _split_waits(bir_bytes):
    bir = orjson.loads(bir_bytes)
    fns = bir.get("functions")
    if fns is None and "modules" in bir:
        fns = [f for m in bir["modules"] for f in m.get("functions", [])]

    def handle_block(block):
        insts = block.get("instructions", [])
        if insts:
            new_insts = []
            for inst in insts:
                si = inst.get("sync_info")
                waits = (si or {}).get("on_wait") or []
                if len(waits) > _MAXW:
                    excess = waits[: len(waits) - _MAXW]
                    si["on_wait"] = waits[len(waits) - _MAXW:]
                    for ci, w in enumerate(excess):
                        new_insts.append({
                            "name": f"{inst['name']}-wsplit{ci}",
                            "opcode": "NoOp",
                            "engine": inst["engine"],
                            "ins": [],
                            "outs": [],
                            "sync_info": {"on_update": [], "on_wait": [w]},
                            **({"debug": inst["debug"]} if "debug" in inst else {}),
                        })
                new_insts.append(inst)
            block["instructions"] = new_insts
        for sub in block.get("blocks", []) or []:
            handle_block(sub)

    for fn in fns or []:
        for blk in fn.get("blocks", []) or []:
            handle_block(blk)
    return orjson.dumps(bir)


_patched = False


def _install_patch():
    global _patched
    if _patched:
        return
    _patched = True
    from concourse import bass_utils, bass2jax

    orig = bass_utils.compile_bir_kernel

    def patched(bir_json, tmpdir, neff_name="file.neff"):
        if isinstance(bir_json, str):
            bir_json = bir_json.encode()
        return orig(_split_waits(bir_json), tmpdir, neff_name=neff_name)

    bass_utils.compile_bir_kernel = patched
    bass2jax.compile_bir_kernel = patched


# ---------------------------------------------------------------------------
# Device kernel builder
# ---------------------------------------------------------------------------

def build_nc(T):
    NBLK = T // TB
    TPB = T * BL
    nc = bass.Bass()

    xTb = nc.declare_dram_parameter("xTb", [NBLK, 128, NKX, 256], BF16, isOutput=False)
    w0T = nc.declare_dram_parameter("w0T", [INP, G4], BF16, isOutput=False)
    wh0T = nc.declare_dram_parameter("wh0T", [H, G4], BF16, isOutput=False)
    w1T = nc.declare_dram_parameter("w1T", [H, G4], BF16, isOutput=False)
    wh1T = nc.declare_dram_parameter("wh1T", [H, G4], BF16, isOutput=False)
    bias0 = nc.declare_dram_parameter("bias0", [128, NM], F32, isOutput=False)
    bias1 = nc.declare_dram_parameter("bias1", [128, NM], F32, isOutput=False)
    whdT = nc.declare_dram_parameter("whdT", [H, AH], BF16, isOutput=False)
    bh = nc.declare_dram_parameter("bh", [AH, 1], F32, isOutput=False)
    h0i = nc.declare_dram_parameter("h0i", [2, 128, NK, BL], BF16, isOutput=False)
    c0i = nc.declare_dram_parameter("c0i", [2, 128, NK, BL], F32, isOutput=False)
    maskrep = nc.declare_dram_parameter("maskrep", [NBLK, 128, TB, NK * BL], BF16, isOutput=False)

    outT = nc.declare_dram_parameter("outT", [NBLK, AH, TB * BL], F32, isOutput=True)
    hfin = nc.declare_dram_parameter("hfin", [2, 128, NK * BL], F32, isOutput=True)
    cfin = nc.declare_dram_parameter("cfin", [2, 128, NK * BL], F32, isOutput=True)

    a0d = nc.dram_tensor("a0d", [NBLK, NM, 128, TB * BL], BF16)
    a1d = nc.dram_tensor("a1d", [NBLK, NM, 128, TB * BL], BF16)
    h0d = nc.dram_tensor("h0d", [NBLK, 128, TB, NK * BL], BF16)
    h1d = nc.dram_tensor("h1d", [NBLK, 128, TB, NK * BL], BF16)

    # ---------------- phase 1: A0 = W_ih0 @ x^T (+bias) ----------------
    def bulk_in_proj(wT, nkk, src_is_x, bias, adst):
        with tile.TileContext(nc) as tc:
            with tc.tile_pool(name="wp", bufs=1) as wp, \
                 tc.tile_pool(name="bp", bufs=1) as bp, \
                 tc.tile_pool(name="mv", bufs=3) as mv, \
                 tc.tile_pool(name="ps", bufs=4, space="PSUM") as psp, \
                 tc.tile_pool(name="ast", bufs=4) as astp:
                wsb = wp.tile([128, nkk, G4], BF16)
                nc.sync.dma_start(out=wsb, in_=wT.rearrange("(k p) g -> p k g", p=128))
                bsb = bp.tile([128, NM], F32)
                nc.sync.dma_start(out=bsb, in_=bias[:])
                with tc.For_i(0, NBLK, 1) as i:
                    if src_is_x:
                        rhs = mv.tile([128, nkk, 256], BF16)
                        nc.sync.dma_start(out=rhs, in_=xTb[ds(i, 1)][0])
                    else:
                        rhs = mv.tile([128, TB, NK * BL], BF16)
                        nc.sync.dma_start(out=rhs, in_=h0d[ds(i, 1)][0])
                    astblk = astp.tile([128, NM, TB * BL], BF16)
                    for m in range(NM):
                        ps = psp.tile([128, TB * BL], F32)
                        for k in range(nkk):
                            if src_is_x:
                                rk = rhs[:, k, :]
                            else:
                                rk = rhs[:, :, ds(k * BL, BL)]
                            nc.tensor.matmul(ps, wsb[:, k, ds(m * 128, 128)], rk,
                                             start=(k == 0), stop=(k == nkk - 1))
                        nc.vector.tensor_scalar_add(astblk[:, m], ps, bsb[:, ds(m, 1)])
                    nc.sync.dma_start(out=adst[ds(i, 1)][0].rearrange("m p n -> p m n"), in_=astblk)

    bulk_in_proj(w0T[:], NKX, True, bias0[:], a0d)

    # ---------------- recurrence over one layer ----------------
    def recurrence(layer, whT, asrc, hdst):
        with tile.TileContext(nc) as tc:
            with tc.tile_pool(name="wp", bufs=1) as wp, \
                 tc.tile_pool(name="st", bufs=1) as stp, \
                 tc.tile_pool(name="ab", bufs=2) as abp, \
                 tc.tile_pool(name="mb", bufs=2) as mbp, \
                 tc.tile_pool(name="ho", bufs=2) as hop, \
                 tc.tile_pool(name="sm", bufs=4) as smp, \
                 tc.tile_pool(name="ps", bufs=2, space="PSUM") as psp:
                wsb = wp.tile([128, NK, G4], BF16)
                nc.sync.dma_start(out=wsb, in_=whT.rearrange("(k p) g -> p k g", p=128))
                h_st = stp.tile([128, NK, BL], BF16)
                c_st = stp.tile([128, NK, BL], F32)
                nc.sync.dma_start(out=h_st, in_=h0i[layer])
                nc.sync.dma_start(out=c_st, in_=c0i[layer])

                with tc.For_i(0, NBLK, 1) as i:
                    ablk = abp.tile([128, NM, TB, BL], BF16)
                    nc.sync.dma_start(out=ablk, in_=asrc[ds(i, 1)][0].rearrange("m p (t b) -> p m t b", t=TB))
                    mblk = mbp.tile([128, TB, NK, BL], BF16)
                    nc.sync.dma_start(out=mblk, in_=maskrep[ds(i, 1)][0].rearrange("p t (j b) -> p t j b", j=NK))
                    hout = hop.tile([128, TB, NK, BL], BF16)
                    for u in range(TB):
                        hin = smp.tile([128, NK, BL], BF16, tag="hin")
                        nc.vector.tensor_mul(hin, h_st, mblk[:, u])
                        cin = smp.tile([128, NK, BL], F32, tag="cin")
                        nc.vector.tensor_mul(cin, c_st, mblk[:, u])
                        ps = psp.tile([128, NM, BL], F32)
                        for m in range(NM):
                            for k in range(NK):
                                nc.tensor.matmul(ps[:, m], wsb[:, k, ds(m * 128, 128)],
                                                 hin[:, k], start=(k == 0), stop=(k == NK - 1))
                        gsb = smp.tile([128, NM, BL], F32, tag="gsb")
                        nc.vector.tensor_add(gsb, ps, ablk[:, :, u, :])
                        sif = smp.tile([128, 2 * NK, BL], F32, tag="sif")
                        nc.scalar.activation(sif, gsb[:, 0:2 * NK, :], AF.Sigmoid)
                        gt = smp.tile([128, NK, BL], F32, tag="gt")
                        nc.scalar.activation(gt, gsb[:, 2 * NK:3 * NK, :], AF.Tanh)
                        os_ = smp.tile([128, NK, BL], F32, tag="os")
                        nc.scalar.activation(os_, gsb[:, 3 * NK:4 * NK, :], AF.Sigmoid)
                        t1 = smp.tile([128, NK, BL], F32, tag="t1")
                        nc.vector.tensor_mul(t1, sif[:, NK:2 * NK, :], cin)
                        t2 = smp.tile([128, NK, BL], F32, tag="t2")
                        nc.vector.tensor_mul(t2, sif[:, 0:NK, :], gt)
                        nc.vector.tensor_add(c_st, t1, t2)
                        tc_ = smp.tile([128, NK, BL], F32, tag="tc")
                        nc.scalar.activation(tc_, c_st, AF.Tanh)
                        nc.vector.tensor_mul(h_st, os_, tc_)
                        nc.scalar.copy(hout[:, u], h_st)
                    nc.sync.dma_start(out=hdst[ds(i, 1)][0].rearrange("p t (j b) -> p t j b", j=NK), in_=hout)

                # final states out
                hf = stp.tile([128, NK * BL], F32)
                nc.vector.tensor_copy(hf, h_st.rearrange("p j b -> p (j b)"))
                nc.sync.dma_start(out=hfin[layer], in_=hf)
                nc.sync.dma_start(out=cfin[layer], in_=c_st.rearrange("p j b -> p (j b)"))

    recurrence(0, wh0T[:], a0d, h0d)
    bulk_in_proj(w1T[:], NK, False, bias1[:], a1d)
    recurrence(1, wh1T[:], a1d, h1d)

    # ---------------- heads ----------------
    with tile.TileContext(nc) as tc:
        with tc.tile_pool(name="wp", bufs=1) as wp, \
             tc.tile_pool(name="mv", bufs=3) as mv, \
             tc.tile_pool(name="ps", bufs=4, space="PSUM") as psp, \
             tc.tile_pool(name="ob", bufs=4) as obp:
            wsb = wp.tile([128, NK, AH], BF16)
            nc.sync.dma_start(out=wsb, in_=whdT.rearrange("(k p) a -> p k a", p=128))
            bsb = wp.tile([AH, 1], F32)
            nc.sync.dma_start(out=bsb, in_=bh[:])
            with tc.For_i(0, NBLK, 1) as i:
                hblk = mv.tile([128, TB, NK * BL], BF16)
                nc.sync.dma_start(out=hblk, in_=h1d[ds(i, 1)][0])
                ps = psp.tile([AH, TB * BL], F32)
                for k in range(NK):
                    nc.tensor.matmul(ps, wsb[:, k, :],
                                     hblk[:, :, ds(k * BL, BL)],
                                     start=(k == 0), stop=(k == NK - 1))
                osb = obp.tile([AH, TB * BL], F32)
                nc.vector.tensor_scalar_add(osb, ps, bsb[:, 0:1])
                nc.sync.dma_start(out=outT[ds(i, 1)][0], in_=osb)

    return nc


# ---------------------------------------------------------------------------
# Host-side sharding / gathering
# ---------------------------------------------------------------------------

def _prep_core_inputs(c, T, x, done, h0, c0, W_ih0, W_hh0, b_ih0, b_hh0,
                      W_ih1, W_hh1, b_ih1, b_hh1, W_actor, b_actor, W_critic, b_critic):
    NBLK = T // TB
    bs = slice(c * BL, (c + 1) * BL)
    bf = ml_dtypes.bfloat16

    xs = x[:, bs, :]                                   # [T, BL, IN]
    xT = np.zeros((INP, T * BL), np.float32)
    xT[:IN] = xs.transpose(2, 0, 1).reshape(IN, T * BL)
    xTb = np.ascontiguousarray(
        xT.reshape(NKX, 128, NBLK, TB * BL).transpose(2, 1, 0, 3)
    ).astype(bf)                                       # [NBLK, 128, NKX, 256]

    w0T = np.zeros((INP, G4), np.float32)
    w0T[:IN] = W_ih0.T
    wh0T = W_hh0.T.astype(bf)
    w1T = W_ih1.T.astype(bf)
    wh1T = W_hh1.T.astype(bf)

    bias0 = (b_ih0 + b_hh0).reshape(NM, 128).T.astype(np.float32)
    bias1 = (b_ih1 + b_hh1).reshape(NM, 128).T.astype(np.float32)

    whd = np.concatenate([W_actor, W_critic], 0)       # [19, H]
    whdT = whd.T.astype(bf)                            # [H, 19]
    bhv = np.concatenate([b_actor, b_critic])[:, None].astype(np.float32)

    def packT(v):                                      # [BL, H] -> [128, NK, BL]
        return np.ascontiguousarray(v.T.reshape(NK, 128, BL).transpose(1, 0, 2))

    h0i = np.stack([packT(h0[l, bs]) for l in range(2)]).astype(bf)
    c0i = np.stack([packT(c0[l, bs]) for l in range(2)]).astype(np.float32)

    m = (1.0 - done[:, bs].astype(np.float32))         # [T, BL]
    mrep = np.broadcast_to(
        m.reshape(NBLK, TB, BL)[:, None, :, None, :], (NBLK, 128, TB, NK, BL)
    ).reshape(NBLK, 128, TB, NK * BL).astype(bf)

    return {
        "xTb": xTb, "w0T": w0T.astype(bf), "wh0T": wh0T, "w1T": w1T, "wh1T": wh1T,
        "bias0": bias0, "bias1": bias1, "whdT": whdT, "bh": bhv,
        "h0i": h0i, "c0i": np.ascontiguousarray(c0i), "maskrep": np.ascontiguousarray(mrep),
    }


_cache = {}


def kernel(**inputs):
    _install_patch()
    inputs = {k: np.asarray(v) for k, v in inputs.items()}
    T = inputs["x"].shape[0]
    NBLK = T // TB

    if T not in _cache:
        _cache[T] = build_nc(T)
    nc = _cache[T]

    in_maps = [_prep_core_inputs(c, T, **inputs) for c in range(NCORES)]
    res = run_bass_kernel_spmd(nc, in_maps, core_ids=list(range(NCORES)),
                               trace=bool(int(os.environ.get("KERNEL_TRACE", "0"))))
    globals()["_last_res"] = res

    out = np.empty((T, B, AH), np.float32)
    h_T = np.empty((2, B, H), np.float32)
    c_T = np.empty((2, B, H), np.float32)
    for c in range(NCORES):
        r = res.results[c]
        bs = slice(c * BL, (c + 1) * BL)
        o = r["outT"].reshape(NBLK, AH, TB, BL)        # [NBLK, 19, TB, BL]
        out[:, bs, :] = o.transpose(0, 2, 3, 1).reshape(T, BL, AH)
        for l in range(2):
            h_T[l, bs] = r["hfin"][l].reshape(128, NK, BL).transpose(2, 1, 0).reshape(BL, H)
            c_T[l, bs] = r["cfin"][l].reshape(128, NK, BL).transpose(2, 1, 0).reshape(BL, H)
    return out.reshape(T * B, AH), h_T, c_T
